# revision 67
# baseline (speedup 1.0000x reference)

# Causal self-attention (B=8, T=1024, C=768, H=12) on 8 trn2 NeuronCores.
# Strategy: pure data parallelism — one batch element per core. Each core runs
# a fused QKV -> causal attention -> c_proj kernel written in Bass/Tile.
#
# Per-core layout (T=1024, C=768, H=12, D=64), bf16 compute with fp8 Q/K proj:
#   - host pre-transposes x to xT [C, T] bf16; weights bf16.
#   - Q/K projection runs in fp8(e4m3) DoubleRow perf mode: contraction pairs
#     channels (c, c+128) per partition so 3 matmuls replace 6 at double MAC
#     rate. fp8 quantization of x/wqk only perturbs attention *logits* (~1%
#     in the weights after softmax), which washes out over the softmax
#     average — it never touches the value path, so the output error stays
#     far inside the 2e-2 budget. V/c_proj stay bf16 (their quantization
#     error would hit the output directly).
#   - q,k produced transposed (qT/kT [D,T] per head, 2 heads per 128-row
#     tile). scoresT[tk,tq] = kT.T @ qT (contraction D=64, two heads packed
#     via PE row groups 0/64). Causal: fully-masked 128-col blocks are
#     skipped exactly; the diagonal block gets -1e30 added via an
#     ident @ negtri matmul, exp maps it to 0.
#   - softmax without max-subtraction (logits bounded); exp on ScalarE.
#   - PV is flipped: pv[tq, d|denom] = ex_blk.T @ [v | 1] — the softmax
#     denominator accumulates in the same matmul (ones column). Two tq
#     blocks x two heads share one PSUM tile [128, 4*65]; one strided DVE
#     reciprocal + one stride-0-broadcast multiply normalize all four
#     groups (coarse DVE ops keep per-block semaphore hops off the HW
#     critical path).
#   - normalized yb4 [tq, 2x128] bf16 goes through XBAR dma transposes
#     (SP-issued) to yT[c2, tq] — the PE does no transpose matmuls and the
#     PV->c_proj chain never blocks the tensor engine (except p5/i1, where
#     a PE transpose + DVE copy shortens the drain tail).
#   - emission is software-pipelined: qk-proj of head-pair p+1, the V
#     projection (p=0), and c_proj tiles (p=5) are queued as filler units and
#     interleaved between score tiles / PV chains, so the PE stream has
#     independent work wherever the exp (ScalarE) or normalize (DVE) chain
#     would otherwise stall it. This also keeps the tensor engine's DVFS
#     ramp from resetting.
#   - all load DMAs issue from SP/Pool queues (never ScalarE, which is
#     saturated by exp); stores + v-ones memsets go to the Pool engine.
#   - v-bias and proj-bias folded on host: out = (Pnorm @ v~) @ w_proj +
#     (b_v @ w_proj + b_proj); 1/sqrt(D) and b_q folded into w_attn's q
#     columns (exact /8). out stored bf16, cast to f32 on host.

import sys

sys.path.insert(0, "/opt/trn_rl_repo")

import numpy as np

import concourse.bass as bass
import concourse.bacc as bacc
import concourse.mybir as mybir
import concourse.tile as tile
from concourse.vector_clock import ScopedClock

B, T, C, H = 8, 1024, 768, 12
D = C // H  # 64
NCORES = 8
F32 = mybir.dt.float32
BF16 = mybir.dt.bfloat16
FP8 = mybir.dt.float8e4
VE = 65  # v columns per head incl. the ones column

FP8_QK = True  # fp8 DoubleRow Q/K projection
# bf16 recompute of qT/kT block 0 (rows 0..127 see little softmax averaging,
# so fp8 logit noise doesn't wash out there). Costs the wqk bf16 load + 12
# fixup chains; without it rel err is ~1.1e-2 (vs 4e-3), still under the
# 2e-2 gate.
FIXUP_B0 = False

# ---------------------------------------------------------------------------
# This walrus build accepts only one sync wait per CTRL instruction; the Tile
# tail drain aggregates one wait per logical processor. Split the excess waits
# onto dedicated NOPs ahead of the drain.
_MAX_WAITS = 1
_PATCHED = False


def _patch_tile_drain():
    global _PATCHED
    if _PATCHED:
        return
    _PATCHED = True

    def _drain_and_barrier(self, tick_clock, wait_clock):
        nc = self.nc
        bb = nc.cur_bb.bb
        idx_before = len(bb.instructions)
        drain_inst = nc.sync.drain()
        wait_clock.add_sem_waits(
            drain_inst.ins, ScopedClock({None: tick_clock.global_clock})
        )
        si = drain_inst.ins.sync_info
        if si is not None and si.on_wait and len(si.on_wait) > _MAX_WAITS:
            waits = list(si.on_wait)
            si.on_wait = waits[:_MAX_WAITS]
            extra = waits[_MAX_WAITS:]
            nops = []
            for i in range(0, len(extra), _MAX_WAITS):
                nop = nc.sync.nop(hint=f"drain_wait_spill_{i}", nofuse=True)
                nop.ins.sync_info = mybir.SyncInfo(
                    on_wait=extra[i : i + _MAX_WAITS], on_update=[]
                )
                nops.append(nop.ins)
            insts = bb.instructions
            tail = list(insts[idx_before:])
            assert tail[0] is drain_inst.ins
            del insts[idx_before:]
            for n in nops:
                insts.append(n)
            for t in tail:
                insts.append(t)
        nc.all_engine_barrier()
        popped = nc._tile_sem_poison_stack.pop()
        assert popped is self._sem_poison
        nc.clear_and_free_semaphores(list(self.sems.allocated().values()))
        nc.all_engine_barrier()

    tile.TileContext._drain_and_barrier = _drain_and_barrier


# ---------------------------------------------------------------------------


def build_program(loop_n=None, phases="lvqac"):
    _patch_tile_drain()
    nc = bacc.Bacc("TRN2", target_bir_lowering=False, debug=False)

    io = dict(
        xT=nc.dram_tensor("xT", [C, T], BF16, kind="ExternalInput").ap(),
        # packed bf16 weights [wqk | wv | wP]; in fp8 mode the wqk part is
        # only read by the tq/tk block-0 fixup chains.
        wall=nc.dram_tensor("wall", [C, 4 * C], BF16, kind="ExternalInput").ap(),
        bqk=nc.dram_tensor("bqk", [128, 12], F32, kind="ExternalInput").ap(),
        btot=nc.dram_tensor("btot", [1, C], BF16, kind="ExternalInput").ap(),
        ident=nc.dram_tensor("ident", [128, 128], BF16, kind="ExternalInput").ap(),
        negtri=nc.dram_tensor("negtri", [128, 128], BF16, kind="ExternalInput").ap(),
        out=nc.dram_tensor("out", [T, C], BF16, kind="ExternalOutput").ap(),
    )
    if FP8_QK:
        # paired layouts for DoubleRow: [128, 2, N] with (p, i) -> c=256g+128i+p
        io["wqk8"] = nc.dram_tensor(
            "wqk8", [3 * 128, 2 * 2 * C], FP8, kind="ExternalInput"
        ).ap()
        io["xT8"] = nc.dram_tensor(
            "xT8", [3 * 128, 2 * T], FP8, kind="ExternalInput"
        ).ap()
        io["wv8"] = nc.dram_tensor(
            "wv8", [3 * 128, 2 * C], FP8, kind="ExternalInput"
        ).ap()

    with tile.TileContext(nc) as tc:
        if loop_n is None:
            _emit_body(nc, tc, io, phases)
        else:
            with tc.For_i(0, loop_n, 1):
                _emit_body(nc, tc, io, phases)
    nc.compile()
    return nc


def _emit_body(nc, tc, io, phases="lvqac"):
    EXP = mybir.ActivationFunctionType.Exp
    DR = mybir.MatmulPerfMode.DoubleRow

    with (
        tc.tile_pool(name="persist", bufs=1) as persist,
        tc.tile_pool(name="qk", bufs=3) as qk_pool,
        tc.tile_pool(name="ex", bufs=13) as ex_pool,
        tc.tile_pool(name="yb", bufs=3) as yb_pool,
        tc.tile_pool(name="rb", bufs=3) as rb_pool,
        tc.tile_pool(name="ob", bufs=2) as ob_pool,
        tc.tile_pool(name="sc_ps", bufs=2, space="PSUM") as sc_ps,
        tc.tile_pool(name="pj_ps", bufs=2, space="PSUM") as pj_ps,
        tc.tile_pool(name="pv_ps", bufs=2, space="PSUM") as pv_ps,
    ):
        # ---- persistent SBUF tensors + loads ------------------------
        # SP queue: q/k-proj operands first (first compute), then xT.
        # Pool queue: small consts first, then wv / wP.
        if FP8_QK:
            # arrival-ordered loads: the (q,th1)/(k,th1) chains fire first
            # (p0 emits its i=1 score tiles before the block-0 fixup lands),
            # so ship w8 q-halves + x8 th1-halves ahead of the rest.
            wqk8_sb, xT8_sb = [], []
            for g in range(3):
                w_ = persist.tile([128, 24 * 128], FP8, tag=f"wqk8_{g}", name=f"w8{g}")
                wqk8_sb.append(w_)
                x_ = persist.tile([128, 2 * T], FP8, tag=f"xT8_{g}", name=f"x8{g}")
                xT8_sb.append(x_)
            for g in range(3):
                nc.sync.dma_start(
                    out=wqk8_sb[g][:],
                    in_=io["wqk8"][128 * g : 128 * (g + 1), :],
                )
                nc.sync.dma_start(
                    out=xT8_sb[g][:],
                    in_=io["xT8"][128 * g : 128 * (g + 1), :],
                )
        bqk_sb = persist.tile([128, 12], F32, tag="bqk", name="bqk")
        nc.gpsimd.dma_start(out=bqk_sb[:], in_=io["bqk"][:, :])
        ident_sb = persist.tile([128, 128], BF16, tag="ident", name="ident")
        nc.gpsimd.dma_start(out=ident_sb[:], in_=io["ident"][:, :])
        negtri_sb = persist.tile([128, 128], BF16, tag="negtri", name="negtri")
        nc.gpsimd.dma_start(out=negtri_sb[:], in_=io["negtri"][:, :])
        btot_sb = persist.tile([128, C], BF16, tag="btot", name="btot")
        btot_bcast = bass.AP(
            tensor=io["btot"].tensor,
            offset=io["btot"].offset,
            ap=[[0, 128], [1, C]],
        )
        nc.gpsimd.dma_start(out=btot_sb[:], in_=btot_bcast)

        # one packed bf16 weight wall per 128-channel block: [wqk | wv | wP],
        # loaded in two column waves so the early-needed wqk part doesn't
        # wait for wv/wP bytes — 12 DMA issues instead of 18.
        wall_sb, wqk_sb, wv_sb, wp_sb = [], [], [], []
        for c in range(6):
            t_ = persist.tile([128, 4 * C], BF16, tag=f"wall{c}", name=f"wall{c}")
            if FIXUP_B0 or not FP8_QK:
                eng = nc.gpsimd if FP8_QK else nc.sync
                eng.dma_start(
                    out=t_[:, 0 : 2 * C],
                    in_=io["wall"][128 * c : 128 * (c + 1), 0 : 2 * C],
                )
            wall_sb.append(t_)
            wqk_sb.append(t_[:, 0 : 2 * C])
            wv_sb.append(t_[:, 2 * C : 3 * C])
            wp_sb.append(t_[:, 3 * C : 4 * C])
        for c in range(6):
            nc.gpsimd.dma_start(
                out=wall_sb[c][:, 2 * C : 4 * C],
                in_=io["wall"][128 * c : 128 * (c + 1), 2 * C : 4 * C],
            )

        if FP8_QK:
            # only xT columns 0:128 are needed in bf16 (block-0 fixup rhs +
            # vproj t=0 stationary); vproj t>=1 runs fp8 DoubleRow off xT8.
            xT_sb = []
            for c in range(6):
                t_ = persist.tile([128, 128], BF16, tag=f"xT{c}", name=f"xT{c}")
                nc.sync.dma_start(
                    out=t_[:], in_=io["xT"][128 * c : 128 * (c + 1), 0:128]
                )
                xT_sb.append(t_)
            wv8_sb = []
            for g in range(3):
                t_ = persist.tile([128, 2 * C], FP8, tag=f"wv8_{g}", name=f"wv8{g}")
                nc.gpsimd.dma_start(
                    out=t_[:], in_=io["wv8"][128 * g : 128 * (g + 1), :]
                )
                wv8_sb.append(t_)
        else:
            xT_sb = []
            for c in range(6):
                t_ = persist.tile([128, T], BF16, tag=f"xT{c}", name=f"xT{c}")
                nc.sync.dma_start(
                    out=t_[:], in_=io["xT"][128 * c : 128 * (c + 1), :]
                )
                xT_sb.append(t_)
        ones_sb = persist.tile([1, 128], BF16, tag="ones", name="ones")
        nc.gpsimd.memset(ones_sb[:], 1.0)

        # v with a ones column per head: [128, 12*65]
        v_sb = [
            persist.tile([128, H * VE], BF16, tag=f"v{t}", name=f"v{t}")
            for t in range(8)
        ]
        for t in range(8):
            v3 = v_sb[t][:].rearrange("p (h c) -> p h c", h=H)
            nc.gpsimd.memset(v3[:, :, D : D + 1], 1.0)
        yT_sb = [
            persist.tile([128, T], BF16, tag=f"yT{p}", name=f"yT{p}")
            for p in range(6)
        ]

        # ---- filler-unit machinery ---------------------------------
        # A unit is [fn, done]. Units are popped FIFO between score tiles /
        # PV chains to keep the PE stream dense; force() runs a unit list
        # early when a dependency requires it.
        queue = []

        def push(fn):
            u = [fn, False]
            queue.append(u)
            return u

        def run(u):
            if not u[1]:
                u[1] = True
                u[0]()

        def pop_fillers(n):
            k = 0
            while queue and k < n:
                u = queue.pop(0)
                if u[1]:
                    continue
                run(u)
                k += 1

        def force(units):
            for u in units:
                run(u)

        # ---- q/k projection units (per head-pair p) ----------------
        qk_tiles = {}

        def qkproj_units(p, th_order=(0, 1)):
            units = []
            qkt = {}
            for kind, jt in (("q", p), ("k", 6 + p)):
                qkt[kind] = qk_pool.tile(
                    [128, T], BF16, tag=f"qk_{kind}", name=f"qk{kind}{p}"
                )
            qk_tiles[p] = (qkt["q"], qkt["k"])

            for th in th_order:
                for kind, jt in (("q", p), ("k", 6 + p)):
                    box = {}
                    if FP8_QK:
                        # operands are pre-scaled (x*8, w*16) to stay out of
                        # e4m3 subnormals; descale here, fused with the bias.
                        descale = 1.0 / (128.0 * (8.0 if kind == "q" else 1.0))

                        def a_(kind=kind, jt=jt, th=th, box=box):
                            ps = pj_ps.tile([128, 512], F32, tag="pj", name="pjq")
                            box["ps"] = ps
                            for g in range(2):
                                w3 = wqk8_sb[g][:].rearrange(
                                    "p (jt two n) -> p jt two n", jt=12, two=2
                                )
                                x3 = xT8_sb[g][:].rearrange(
                                    "p (two t) -> p two t", two=2
                                )
                                nc.tensor.matmul(
                                    ps[:, :],
                                    lhsT=w3[:, jt, :, :],
                                    rhs=x3[:, :, 512 * th : 512 * (th + 1)],
                                    start=(g == 0),
                                    stop=False,
                                    perf_mode=DR,
                                )

                        def b_(kind=kind, jt=jt, th=th, box=box, descale=descale):
                            ps = box["ps"]
                            g = 2
                            w3 = wqk8_sb[g][:].rearrange(
                                "p (jt two n) -> p jt two n", jt=12, two=2
                            )
                            x3 = xT8_sb[g][:].rearrange(
                                "p (two t) -> p two t", two=2
                            )
                            nc.tensor.matmul(
                                ps[:, :],
                                lhsT=w3[:, jt, :, :],
                                rhs=x3[:, :, 512 * th : 512 * (th + 1)],
                                start=False,
                                stop=True,
                                perf_mode=DR,
                            )
                            nc.vector.tensor_scalar(
                                qkt[kind][:, 512 * th : 512 * (th + 1)],
                                ps[:, :],
                                descale,
                                bqk_sb[:, jt : jt + 1],
                                mybir.AluOpType.mult,
                                mybir.AluOpType.add,
                            )

                    else:

                        def a_(kind=kind, jt=jt, th=th, box=box):
                            ps = pj_ps.tile([128, 512], F32, tag="pj", name="pjq")
                            box["ps"] = ps
                            for c in range(3):
                                nc.tensor.matmul(
                                    ps[:, :],
                                    lhsT=wqk_sb[c][:, 128 * jt : 128 * (jt + 1)],
                                    rhs=xT_sb[c][:, 512 * th : 512 * (th + 1)],
                                    start=(c == 0),
                                    stop=False,
                                )

                        def b_(kind=kind, jt=jt, th=th, box=box):
                            ps = box["ps"]
                            for c in range(3, 6):
                                nc.tensor.matmul(
                                    ps[:, :],
                                    lhsT=wqk_sb[c][:, 128 * jt : 128 * (jt + 1)],
                                    rhs=xT_sb[c][:, 512 * th : 512 * (th + 1)],
                                    start=False,
                                    stop=(c == 5),
                                )
                            nc.vector.tensor_scalar_add(
                                qkt[kind][:, 512 * th : 512 * (th + 1)],
                                ps[:, :],
                                bqk_sb[:, jt : jt + 1],
                            )

                    units.append(push(a_))
                    units.append(push(b_))
            if FP8_QK and FIXUP_B0:
                # bf16 fixup of tq/tk block 0: rows 0..127 attend to few
                # keys, so fp8 logit noise doesn't wash out there. Emitted
                # after all th chains so the PE doesn't stall on the
                # (late-arriving) bf16 weight tiles.
                for kind, jt in (("q", p), ("k", 6 + p)):

                    def fx_(kind=kind, jt=jt):
                        ps = pj_ps.tile([128, 128], F32, tag="pj", name="pjf")
                        for c in range(6):
                            nc.tensor.matmul(
                                ps[:, :],
                                lhsT=wqk_sb[c][:, 128 * jt : 128 * (jt + 1)],
                                rhs=xT_sb[c][:, 0:128],
                                start=(c == 0),
                                stop=(c == 5),
                            )
                        nc.vector.tensor_scalar_add(
                            qkt[kind][:, 0:128],
                            ps[:, :],
                            bqk_sb[:, jt : jt + 1],
                        )

                    units.append(push(fx_))
            return units

        # ---- V projection units (per t block, 2 half-chunks) -------
        # t=0 runs bf16 (rows 0..127 read only v block 0, where fp8 noise
        # would not wash out); t>=1 runs fp8 DoubleRow off xT8/wv8 and
        # descales by 1/128 in the PSUM->SBUF copy.
        def vproj_units(t):
            units = []
            for half in range(2):

                def fn(t=t, half=half):
                    vps = pj_ps.tile([128, 384], F32, tag="pj", name="pjv")
                    n0 = 384 * half
                    if FP8_QK and t > 0:
                        for g in range(3):
                            x3 = xT8_sb[g][:].rearrange(
                                "p (two n) -> p two n", two=2
                            )
                            w3 = wv8_sb[g][:].rearrange(
                                "p (two n) -> p two n", two=2
                            )
                            nc.tensor.matmul(
                                vps[:, :],
                                lhsT=x3[:, :, 128 * t : 128 * (t + 1)],
                                rhs=w3[:, :, n0 : n0 + 384],
                                start=(g == 0),
                                stop=(g == 2),
                                perf_mode=DR,
                            )
                    else:
                        for c in range(6):
                            nc.tensor.matmul(
                                vps[:, :],
                                lhsT=xT_sb[c][:, 128 * t : 128 * (t + 1)],
                                rhs=wv_sb[c][:, n0 : n0 + 384],
                                start=(c == 0),
                                stop=(c == 5),
                            )
                    v3 = v_sb[t][:].rearrange("p (h c) -> p h c", h=H)
                    vp3 = vps[:].rearrange("p (h c) -> p h c", h=6)
                    if FP8_QK and t > 0:
                        nc.vector.tensor_scalar_mul(
                            v3[:, 6 * half : 6 * half + 6, 0:D],
                            vp3[:, :, :],
                            1.0 / 128.0,
                        )
                    else:
                        nc.vector.tensor_copy(
                            v3[:, 6 * half : 6 * half + 6, 0:D], vp3[:, :, :]
                        )

                units.append(push(fn))
            return units

        # ---- c_proj units (per t block, 2 half-chunks) -------------
        def cproj_units(t):
            units = []
            box = {}
            for half in range(2):

                def fn(t=t, half=half, box=box):
                    cps = pj_ps.tile([128, 384], F32, tag="pj", name="pjc")
                    n0 = 384 * half
                    for c in range(6):
                        nc.tensor.matmul(
                            cps[:, :],
                            lhsT=yT_sb[c][:, 128 * t : 128 * (t + 1)],
                            rhs=wp_sb[c][:, n0 : n0 + 384],
                            start=(c == 0),
                            stop=(c == 5),
                        )
                    if half == 0:
                        box["ob"] = ob_pool.tile(
                            [128, C], BF16, tag="ob", name="ob"
                        )
                    ob = box["ob"]
                    nc.vector.tensor_add(
                        ob[:, n0 : n0 + 384], cps[:, :], btot_sb[:, n0 : n0 + 384]
                    )
                    # one store per t block (HW charges per DMA issue and
                    # per completion; SP queue is idle by now)
                    if half == 1:
                        nc.sync.dma_start(
                            out=io["out"][128 * t : 128 * (t + 1), :],
                            in_=ob[:],
                        )

                units.append(push(fn))
            return units

        # ---- build the unit schedule -------------------------------
        proj_next = {}
        if "q" in phases:
            # th1 first: p0's first score tiles are (i=1, j=7..1)
            p0_units = qkproj_units(0, th_order=(1, 0))
            force(p0_units)  # first compute — emit inline
            if "a" in phases:
                # queued ahead of the vproj units so its DVE bias-adds land
                # before the p0 PV normalize flood (kT/qT of p1 ready early)
                proj_next[1] = qkproj_units(1)
        vunits = [vproj_units(t) for t in range(8)] if "v" in phases else []

        # ---- attention (per head-pair p) ---------------------------
        def attention_p(p):
            qT, kT = qk_tiles[p]
            if 1 <= p < 5 and "q" in phases:
                proj_next[p + 1] = qkproj_units(p + 1)
            ex_map = {}

            def score_tile(i, j):
                r = j - 4 * i
                off = 128 * r if r > 0 else 0
                sc = sc_ps.tile([128, 1024], F32, tag="sc", name="sc")
                sc3 = sc[:].rearrange("p (h c) -> p h c", h=2)
                for h in range(2):
                    nc.tensor.matmul(
                        sc3[:, h, off:512],
                        lhsT=kT[64 * h : 64 * (h + 1), 128 * j : 128 * (j + 1)],
                        rhs=qT[
                            64 * h : 64 * (h + 1), 512 * i + off : 512 * (i + 1)
                        ],
                        tile_position=(64 * h, 0),
                    )
                if r >= 0:
                    for h in range(2):
                        nc.tensor.matmul(
                            sc3[:, h, 128 * r : 128 * (r + 1)],
                            lhsT=ident_sb[:],
                            rhs=negtri_sb[:],
                            start=False,
                            stop=True,
                            skip_group_check=True,
                        )
                ex = ex_pool.tile([128, 1024], BF16, tag="ex", name="ex")
                ex3 = ex[:].rearrange("p (h c) -> p h c", h=2)
                nc.scalar.activation(ex3[:, :, off:512], sc3[:, :, off:512], EXP)
                ex_map[(i, j)] = ex
                pop_fillers(2 if p == 0 else 1)

            def pv_pair(i, mlp):
                """PV + normalize for tq blocks (2*mlp, 2*mlp+1) of half i.

                Both blocks x both heads accumulate into one PSUM tile
                [128, 4*65]; a single strided reciprocal and a single
                stride-0-broadcast multiply normalize all four groups —
                coarse DVE ops keep the per-block semaphore dance off the
                hardware's critical path.
                """
                pvt = pv_ps.tile([128, 4 * VE], F32, tag="pv", name="pv")
                for sub in range(2):
                    ml = 2 * mlp + sub
                    m = 4 * i + ml
                    if "v" in phases:
                        for t in range(m + 1):
                            force(vunits[t])
                    for h in range(2):
                        hd = 2 * p + h
                        col = VE * (2 * sub + h)
                        for j in range(m + 1):
                            nc.tensor.matmul(
                                pvt[:, col : col + VE],
                                lhsT=ex_map[(i, j)][
                                    :, 512 * h + 128 * ml : 512 * h + 128 * (ml + 1)
                                ],
                                rhs=v_sb[j][:, VE * hd : VE * (hd + 1)],
                                start=(j == 0),
                                stop=(j == m),
                            )
                    if sub == 0:
                        pop_fillers(2 if p in (0, 5) else 1)
                rb = rb_pool.tile([128, 4], F32, tag="rb", name="rb")
                pv3 = pvt[:].rearrange("p (g c) -> p g c", g=4)
                nc.vector.reciprocal_approx_fast(rb[:], pv3[:, :, D : D + 1])
                yb4 = yb_pool.tile([128, 256], BF16, tag="yb", name="yb")
                yb3 = yb4[:].rearrange("p (g c) -> p g c", g=4)
                rbb = bass.AP(
                    tensor=rb.tensor,
                    offset=rb.offset,
                    ap=[list(rb.ap[0]), [1, 4], [0, D]],
                )
                nc.vector.tensor_mul(yb3[:, :, :], pv3[:, :, 0:D], rbb)
                for sub in range(2):
                    m = 4 * i + 2 * mlp + sub
                    if p == 5 and i == 1:
                        # tail path: PE transpose + DVE copy has ~2us less
                        # latency than the XBAR round-trip, and the sc psum
                        # slots are idle here (no more score tiles coming).
                        ytp = sc_ps.tile([128, 128], F32, tag="sc", name="ytp")
                        for h in range(2):
                            nc.tensor.matmul(
                                ytp[64 * h : 64 * (h + 1), :],
                                lhsT=yb4[:, 128 * sub + D * h : 128 * sub + D * (h + 1)],
                                rhs=ident_sb[:],
                                tile_position=(0, 64 * h),
                            )
                        nc.vector.tensor_copy(
                            yT_sb[p][:, 128 * m : 128 * (m + 1)], ytp[:, :]
                        )
                    else:
                        nc.sync.dma_start_transpose(
                            yT_sb[p][:, 128 * m : 128 * (m + 1)],
                            yb4[:, 128 * sub : 128 * (sub + 1)],
                        )
                    if p == 5 and "c" in phases:
                        cproj_units(m)
                pop_fillers(2 if p in (0, 5) else 1)

            if p == 0:
                # block-0 of qT/kT waits on the (late) bf16 fixup weights;
                # emit the score tiles that don't read block 0 first so the
                # exp stream starts ~6us earlier.
                for j in range(7, 0, -1):
                    score_tile(1, j)
                for j in range(4):
                    score_tile(0, j)
                score_tile(1, 0)
                for mlp in range(2):
                    pv_pair(0, mlp)
                if p + 1 in proj_next:
                    force(proj_next[p + 1])
                for mlp in range(2):
                    pv_pair(1, mlp)
            else:
                for j in range(4):
                    score_tile(0, j)
                for mlp in range(2):
                    pv_pair(0, mlp)
                for j in range(8):
                    score_tile(1, j)
                if p + 1 in proj_next:
                    force(proj_next[p + 1])
                for mlp in range(2):
                    pv_pair(1, mlp)

        if "a" in phases and "q" in phases:
            for p in range(6):
                attention_p(p)
        elif "c" in phases:
            for t in range(8):
                cproj_units(t)

        # drain any remaining units (c_proj tail, ablation modes)
        pop_fillers(len(queue) + 1)


_NC = None


def _get_nc():
    global _NC
    if _NC is None:
        _NC = build_program()
    return _NC


def make_inputs(x, w_attn, b_attn, w_proj, b_proj):
    """Host-side prep: fold scales/biases, transpose x, build constants."""
    import ml_dtypes

    x = np.asarray(x, dtype=np.float32)
    w_attn = np.asarray(w_attn, dtype=np.float32)
    b_attn = np.asarray(b_attn, dtype=np.float32)
    w_proj = np.asarray(w_proj, dtype=np.float32)
    b_proj = np.asarray(b_proj, dtype=np.float32)

    wA = w_attn.copy()
    wA[:, :C] *= 0.125  # fold 1/sqrt(D)=1/8 into q columns (exact)
    bq = b_attn[:C] * 0.125
    bk = b_attn[C : 2 * C]
    bv = b_attn[2 * C :]
    # bqk[p, j] = bias for feature j*128+p, j in 0..11 (q tiles then k tiles)
    bqk = np.concatenate([bq, bk]).reshape(12, 128).T.copy()
    btot = (
        b_proj.astype(np.float64) + bv.astype(np.float64) @ w_proj.astype(np.float64)
    ).astype(np.float32)[None, :]
    bf = ml_dtypes.bfloat16
    f8 = ml_dtypes.float8_e4m3  # matches mybir.dt.float8e4
    ident = np.eye(128, dtype=np.float32).astype(bf)
    # negtri[tk, tq] = -1e30 where tq < tk (masked), else 0
    negtri = (
        np.where(np.arange(128)[None, :] < np.arange(128)[:, None], -1e30, 0.0)
        .astype(np.float32)
        .astype(bf)
    )
    wall = np.concatenate(
        [wA[:, : 2 * C], wA[:, 2 * C :], w_proj], axis=1
    ).astype(bf)
    shared = {
        "wall": np.ascontiguousarray(wall),
        "bqk": np.ascontiguousarray(bqk),
        "btot": np.ascontiguousarray(btot.astype(bf)),
        "ident": np.ascontiguousarray(ident),
        "negtri": np.ascontiguousarray(negtri),
    }
    if FP8_QK:
        # Unscaled q/k weights *16 and x *8 keep e4m3 values in the normal
        # range (the raw 0.02-sd weights would land in subnormals); the
        # device descales by 1/(128*[8 for q]) in the bias stage, which also
        # restores the 1/sqrt(D) logit scale.
        wqk8f = w_attn[:, : 2 * C] * 16.0
        # DoubleRow pairing, jt-major columns:
        # [g, p, jt*256 + i*128 + n] = src[256g + 128i + p, 128jt + n]
        wqk8 = (
            wqk8f.reshape(3, 2, 128, 12, 128)  # [g, i, p, jt, n]
            .transpose(0, 2, 3, 1, 4)  # [g, p, jt, i, n]
            .reshape(3 * 128, 2 * 2 * C)
        )
        shared["wqk8"] = np.ascontiguousarray(wqk8.astype(f8))
        wv8f = w_attn[:, 2 * C :] * 16.0
        wv8 = (
            wv8f.reshape(3, 2, 128, C).transpose(0, 2, 1, 3).reshape(3 * 128, 2 * C)
        )
        shared["wv8"] = np.ascontiguousarray(wv8.astype(f8))

    in_maps = []
    for b in range(B):
        m = dict(shared)
        xT = x[b].T  # [C, T]
        m["xT"] = np.ascontiguousarray(xT.astype(bf))
        if FP8_QK:
            xT8 = (
                (xT * 8.0)
                .reshape(3, 2, 128, T)
                .transpose(0, 2, 1, 3)
                .reshape(3 * 128, 2 * T)
            )
            m["xT8"] = np.ascontiguousarray(xT8.astype(f8))
        in_maps.append(m)
    return in_maps


def kernel(x, w_attn, b_attn, w_proj, b_proj):
    import time

    from concourse.bass_utils import run_bass_kernel_spmd

    nc = _get_nc()
    in_maps = make_inputs(x, w_attn, b_attn, w_proj, b_proj)
    res = None
    for attempt in range(4):
        try:
            res = run_bass_kernel_spmd(nc, in_maps, list(range(NCORES)))
            break
        except Exception:
            if attempt == 3:
                raise
            time.sleep(30)  # give a wedged NeuronCore time to recover
    out = np.stack([res.results[b]["out"] for b in range(B)], axis=0)
    return out.astype(np.float32)
